# revision 14
# baseline (speedup 1.0000x reference)
"""Trainium2 Bass kernel for a pre-LN multi-head attention block.

Model (per batch b): LayerNorm(x) -> QKV -> 16-head attention (dh=64) ->
output projection + bias.

Sharding over 8 NeuronCores: core c handles batch b = c//2 and head
group hg = c%2 (8 of the 16 heads, all 2048 query rows, full 2048
keys).  LN is duplicated across the pair; QKV/attention are disjoint
per head group; the output projection contracts only this core's 512
inner dims, so each core produces a PARTIAL output and the host adds
the pair's two partials (plus b_out).  No K/V duplication and no
device collectives.

Device-side layout notes:
 - Activations are kept transposed (feature dim on partitions); every
   matmul contracts over the partition axis.
 - Scores are computed directly as S^T [nk, nq] with the two heads of
   a pair row-packed via tile_position (concurrent in the PE array);
   softmax needs no max subtraction (scores ~ N(0,1)), so exp is one
   ScalarE pass and the denominator rides along as a ones-column in
   the PV matmul (M=65).
 - Softmax denominators are collected into one [8, 512] tile per head
   pair and inverted with a single reciprocal_approx_fast (the naive
   per-row DVE reciprocal was ~70us of the baseline).
 - LN: sum(x) on DVE tensor_reduce, sum(x^2) on ScalarE Square+accum
   (both from raw x, var = E[x^2]-mu^2), rstd via
   reciprocal_approx_fast, and (x-mu)*rstd fused in one tensor_scalar.
 - ln_gamma is folded into the QKV weights host-side; the ln_beta
   contribution is a per-output-row bias added at QKV eviction; b_out
   is added host-side.
 - Weights are host-prepacked so each (pair, q/k/v) weight block is a
   single contiguous 256KB DMA (the baseline's 192 small weight DMAs
   were serializing the sync queue).
 - The scores->exp->PV chain is software-pipelined (PV lags one kt)
   with next-pair QKV matmuls interleaved so the PE never waits on
   ScalarE's exp.
"""

import numpy as np
from ml_dtypes import bfloat16

B, N, D = 4, 2048, 1024
HEADS, DH = 16, 64
SCALE = DH ** -0.5
NCORES = 8
NQ = N                      # all 2048 query rows per core
EPS = 1e-5
NT = N // 128               # 16 sequence tiles (LN)
KD = D // 128               # 8 feature tiles
NKT = N // 128              # 16 key tiles
NPAIR = 4                   # head pairs per core (8 heads)
NBLK = NQ // 512            # 4 query blocks of 512
NOB = 12                    # (pair, q/k/v) output row-tiles

_cache = {}


def _build():
    import concourse.bass as bass
    import concourse.mybir as mybir
    import concourse.bacc as bacc
    import concourse.tile as tile
    from concourse.masks import make_identity

    f32 = mybir.dt.float32
    bf16 = mybir.dt.bfloat16
    AX = mybir.AxisListType
    ALU = mybir.AluOpType
    ACTF = mybir.ActivationFunctionType

    nc = bacc.Bacc(
        "TRN2",
        target_bir_lowering=False,
        debug=False,
        enable_asserts=True,
        num_devices=NCORES,
    )

    x_d = nc.dram_tensor("x", [N, D], f32, kind="ExternalInput").ap()
    wq_d = nc.dram_tensor("wpack", [NOB * 128, D], bf16, kind="ExternalInput").ap()
    bias_d = nc.dram_tensor("qkv_bias", [128, NOB], f32, kind="ExternalInput").ap()
    wo_d = nc.dram_tensor("wopack", [NPAIR * 128, D], bf16, kind="ExternalInput").ap()
    out_d = nc.dram_tensor("out", [NQ, D], f32, kind="ExternalOutput").ap()

    with tile.TileContext(nc) as tc:
        with (
            tc.tile_pool(name="persist", bufs=1) as P,
            tc.tile_pool(name="ppool", bufs=1, space="PSUM") as PS,
            tc.tile_pool(name="trans", bufs=1) as T,
        ):
            ident = P.tile([128, 128], bf16, name="ident", tag="ident")
            make_identity(nc, ident)
            eps_t = P.tile([128, 1], f32, name="eps_t", tag="eps_t")
            nc.vector.memset(eps_t, EPS)

            bias_sb = P.tile([128, NOB], f32, name="bias_sb", tag="bias_sb")
            nc.sync.dma_start(bias_sb, bias_d)

            wo_sb = []

            # xnT: transposed normalized activations [d, n] as [128, KD*N]
            xnT = P.tile([128, KD * N], bf16, name="xnT", tag="xnT")
            xnT3 = xnT.rearrange("p (k n) -> p k n", k=KD)
            # normalized attention outputs, transposed: [512 hd, 2048 nq]
            onormT = []
            for k in range(NPAIR):
                onormT.append(
                    P.tile([128, NQ], bf16, name=f"onormT{k}", tag=f"onormT{k}")
                )
            # persistent V_ext buffers [nk, dh | ones]: 4 rotating, the
            # ones-column (idx 64) is initialized once and never rewritten
            ve_bufs = []
            for i in range(4):
                ve = P.tile([128, NKT * 65], bf16, name=f"vx{i}", tag=f"vx{i}")
                ve3 = ve.rearrange("p (k e) -> p k e", e=65)
                nc.vector.memset(ve3[:, :, 64:65], 1.0)
                ve_bufs.append(ve3)

            sq_scr = T.tile([128, D], f32, name="sq_scr", tag="sq", bufs=1)

            # QKV projection work for pair j, packaged as emission
            # closures so it can be interleaved into LN (pair 0) or the
            # previous pair's attention loop.
            def qkv_emitters(j, store):
                ems = []
                reqs = []
                for obi, (key, _) in enumerate(
                    (("qT", 0), ("kT", 1), ("vT", 2))
                ):
                    idx = j * 3 + obi

                    def walloc(j=j, idx=idx, key=key):
                        wt = T.tile(
                            [128, D], bf16, name=f"w{key}{j}",
                            tag=f"w{key}", bufs=2,
                        )
                        nc.sync.dma_start(
                            wt, wq_d[idx * 128:(idx + 1) * 128, :]
                        )
                        store[(key, "w")] = wt
                        store[key] = T.tile(
                            [128, N], bf16, name=f"t{key}{j}", tag=key,
                            bufs=2,
                        )
                    ems.append(walloc)
                    reqs.append(-1)
                    for c in range(N // 512):
                        def chunk(j=j, c=c, idx=idx, key=key):
                            qp = PS.tile(
                                [128, 512], f32, name=f"qp{key}{j}_{c}",
                                tag="work", bufs=2,
                            )
                            wt = store[(key, "w")]
                            for k in range(KD):
                                nc.tensor.matmul(
                                    qp,
                                    lhsT=wt[:, k * 128:(k + 1) * 128],
                                    rhs=xnT3[:, k, c * 512:(c + 1) * 512],
                                    start=(k == 0),
                                    stop=(k == KD - 1),
                                )
                            dcol = store[key][:, c * 512:(c + 1) * 512]
                            nc.vector.tensor_scalar_add(
                                dcol, qp, bias_sb[:, idx:idx + 1]
                            )
                        ems.append(chunk)
                        reqs.append(4 * c + 3)
                # V_ext for the two heads: [nk, dh] blocks per key tile
                for h2 in range(2):
                    for g2 in range(2):
                        def vtr(j=j, h2=h2, g2=g2):
                            p0 = h2 * 64
                            id64 = ident[p0:p0 + 64, p0:p0 + 64]
                            vT_j = store["vT"]
                            ve3 = ve_bufs[(2 * j + h2) % 4]
                            tp = PS.tile(
                                [128, 512], bf16, name=f"vt{j}_{h2}_{g2}",
                                tag="work", bufs=2,
                            )
                            for i8 in range(8):
                                kt = g2 * 8 + i8
                                nc.tensor.transpose(
                                    tp[:, i8 * 64:(i8 + 1) * 64],
                                    vT_j[p0:p0 + 64, kt * 128:(kt + 1) * 128],
                                    id64,
                                )
                            dest = ve3[:, g2 * 8:(g2 + 1) * 8, 0:64]
                            src = tp.rearrange("p (k e) -> p k e", e=64)
                            nc.vector.tensor_copy(dest, src)
                        ems.append(vtr)
                        reqs.append(8 * g2 + 7)
                return ems, reqs

            stores = [dict() for _ in range(NPAIR + 1)]
            # pair-0 weight DMAs go on the sync queue BEFORE the LN x
            # loads so they aren't stuck behind x-buffer WAR waits
            ems0, reqs0 = qkv_emitters(0, stores[0])
            order0 = sorted(range(len(ems0)), key=lambda i: reqs0[i])
            ems0 = [ems0[i] for i in order0]
            reqs0 = [reqs0[i] for i in order0]
            e0i = 0
            while e0i < len(ems0) and reqs0[e0i] < 0:
                ems0[e0i]()
                e0i += 1

            # ---- Phase A: LayerNorm + transpose, two passes over seq
            # tiles so ScalarE's Square and Sqrt calls are batched by ACT
            # table set (each Square<->Sqrt switch costs a 1.3us
            # ACT_TABLE_LOAD; the interleaved version paid it 13x).
            # Pass 1: stats + centered xc (bf16); pass 2: sqrt + scale +
            # transpose.
            for wave in range(2):
                ln_state = {}
                for nt in range(wave * 8, wave * 8 + 8):
                    x_t = T.tile([128, D], f32, name=f"x{nt}", tag="x",
                                 bufs=3)
                    nc.sync.dma_start(x_t, x_d[nt * 128:(nt + 1) * 128, :])
                    ssum = T.tile([128, 1], f32, name=f"ss{nt}", tag="ss",
                                  bufs=3)
                    nc.vector.tensor_reduce(ssum, x_t, AX.X, ALU.add)
                    negmean = T.tile([128, 1], f32, name=f"nm{nt}", tag="nm",
                                     bufs=3)
                    nc.scalar.mul(negmean, ssum, -1.0 / D)
                    # varsum = sum((x - mu)^2) via the ACT bias port
                    varsum = T.tile([128, 1], f32, name=f"vs{nt}", tag="vs",
                                    bufs=3)
                    nc.scalar.activation(sq_scr, x_t, ACTF.Square,
                                         bias=negmean, accum_out=varsum)
                    vareps = T.tile([128, 1], f32, name=f"ve{nt}", tag="vep",
                                    bufs=9)
                    nc.vector.tensor_scalar(
                        vareps, varsum, 1.0 / D, EPS, ALU.mult, ALU.add
                    )
                    xc = T.tile([128, D], bf16, name=f"xc{nt}", tag="xc",
                                bufs=9)
                    nc.vector.tensor_scalar_add(xc, x_t, negmean)
                    ln_state[nt] = (vareps, xc)
                for nt in range(wave * 8, wave * 8 + 8):
                    vareps, xc = ln_state[nt]
                    std = T.tile([128, 1], f32, name=f"st{nt}", tag="st",
                                 bufs=3)
                    nc.scalar.sqrt(std, vareps)
                    rstd = T.tile([128, 1], f32, name=f"rs{nt}", tag="rs",
                                  bufs=3)
                    nc.vector.reciprocal_approx_fast(rstd, std)
                    xhat = T.tile([128, D], bf16, name=f"xh{nt}", tag="xh",
                                  bufs=3)
                    nc.vector.tensor_scalar_mul(xhat, xc, rstd)
                    for g2 in range(2):
                        tp = PS.tile(
                            [128, 512], bf16, name=f"tp{nt}_{g2}",
                            tag="work", bufs=2
                        )
                        for jj in range(4):
                            kd = g2 * 4 + jj
                            nc.tensor.transpose(
                                tp[:, jj * 128:(jj + 1) * 128],
                                xhat[:, kd * 128:(kd + 1) * 128],
                                ident,
                            )
                        dest = xnT3[:, g2 * 4:(g2 + 1) * 4,
                                    nt * 128:(nt + 1) * 128]
                        src = tp.rearrange("p (k n) -> p k n", k=4)
                        if (nt + g2) % 2 == 0:
                            nc.vector.tensor_copy(dest, src)
                        else:
                            nc.scalar.copy(dest, src)
                    # interleave pair-0 QKV emission once its LN inputs
                    # have been emitted (Tile deps follow emission order)
                    while e0i < len(ems0) and reqs0[e0i] <= nt:
                        ems0[e0i]()
                        e0i += 1

            # ---- Phases B+C: per head pair, attention row-packed via
            # tile_position so K stays covered.
            while e0i < len(ems0):
                ems0[e0i]()
                e0i += 1
            for j in range(NPAIR):
                st = stores[j]
                if j == 2:
                    for k in range(NPAIR):
                        t = P.tile([128, D], bf16, name=f"wo{k}", tag=f"wo{k}")
                        nc.sync.dma_start(t, wo_d[k * 128:(k + 1) * 128, :])
                        wo_sb.append(t)
                qT_j, kT_j = st["qT"], st["kT"]
                ve3s = [ve_bufs[(2 * j) % 4], ve_bufs[(2 * j + 1) % 4]]
                if j + 1 < NPAIR:
                    pe_, pr_ = qkv_emitters(j + 1, stores[j + 1])
                    po_ = sorted(range(len(pe_)), key=lambda i: pr_[i])
                    pend = [pe_[i] for i in po_]
                else:
                    pend = []
                pi = 0
                norm_defer = []
                NG = NKT // 2       # 8 groups of 2 key tiles
                for blk in range(NBLK):
                    b0 = blk * 512
                    opss = [
                        PS.tile([65, 512], f32, name=f"ops{2*j}_{blk}",
                                tag="acc0", bufs=1),
                        PS.tile([65, 512], f32, name=f"ops{2*j+1}_{blk}",
                                tag="acc1", bufs=1),
                    ]
                    pts = [None, None]
                    for g in range(NG + 1):
                        if g < NG:
                            # scores for 2 key tiles (4 row-packed MMs)
                            # into one 4-bank psum tile; one exp covers
                            # all of it (halves ScalarE instr overhead)
                            sps = PS.tile(
                                [128, 2048], f32, name=f"s{j}_{blk}_{g}",
                                tag="spair", bufs=1,
                            )
                            for k2 in range(2):
                                kt = 2 * g + k2
                                for h2 in range(2):
                                    p0 = h2 * 64
                                    nc.tensor.matmul(
                                        sps[:, k2 * 1024 + h2 * 512:
                                            k2 * 1024 + (h2 + 1) * 512],
                                        lhsT=kT_j[p0:p0 + 64,
                                                  kt * 128:(kt + 1) * 128],
                                        rhs=qT_j[p0:p0 + 64, b0:b0 + 512],
                                        start=True,
                                        stop=True,
                                        tile_position=(p0, 0),
                                    )
                            pt = T.tile(
                                [128, 2048], bf16, name=f"pt{j}_{blk}_{g}",
                                tag="pt", bufs=3,
                            )
                            nc.scalar.activation(pt, sps, ACTF.Exp, scale=SCALE)
                            pts[g % 2] = pt
                        # interleave next-pair QKV emission across slots
                        it = blk * (NG + 1) + g + 1
                        tot = NBLK * (NG + 1)
                        while pi < len(pend) and pi * tot < len(pend) * it:
                            pend[pi]()
                            pi += 1
                        # PV lags one group so exp never blocks the PE
                        if g >= 1:
                            ptp = pts[(g - 1) % 2]
                            for k2 in range(2):
                                kt = 2 * (g - 1) + k2
                                for h2 in range(2):
                                    nc.tensor.matmul(
                                        opss[h2],
                                        lhsT=ve3s[h2][:, kt, :],
                                        rhs=ptp[:, k2 * 1024 + h2 * 512:
                                                k2 * 1024 + (h2 + 1) * 512],
                                        start=(kt == 0),
                                        stop=(kt == NKT - 1),
                                    )
                    # evict accumulators fast (frees the psum bank); the
                    # denominator row goes to a partition-0 tile so the
                    # fast approx reciprocal can be used (custom DVE ops
                    # misread partition-offset inputs)
                    for h2 in range(2):
                        r = 2 * blk + h2
                        oc = T.tile([64, 512], f32, name=f"oc{j}_{r}",
                                    tag="oc", bufs=8)
                        nc.vector.tensor_copy(oc, opss[h2][0:64, :])
                        rl0 = T.tile([1, 512], f32, name=f"rl0{j}_{r}",
                                     tag="rl0", bufs=8)
                        nc.vector.tensor_copy(rl0, opss[h2][64:65, :])
                        norm_defer.append((h2, blk, oc, rl0))
                while pi < len(pend):
                    pend[pi]()
                    pi += 1
                for h2, blk, oc, rl0 in norm_defer:
                    r = 2 * blk + h2
                    b0 = blk * 512
                    p0 = h2 * 64
                    rl = T.tile([1, 512], f32, name=f"rl{j}_{r}",
                                tag="rl", bufs=2)
                    nc.vector.reciprocal_approx_fast(rl, rl0)
                    rlb = T.tile([64, 512], f32, name=f"rlb{j}_{r}",
                                 tag="rlb", bufs=2)
                    nc.gpsimd.partition_broadcast(rlb, rl, channels=64)
                    nc.vector.tensor_mul(
                        onormT[j][p0:p0 + 64, b0:b0 + 512], oc, rlb
                    )
                norm_defer = []

            # ---- Phase D: output projection [2048 nq, 1024 dm] (partial:
            # this core's 512 inner dims; host adds the pair's partials)
            for nt in range(NQ // 128):
                po = T.tile([128, D], f32, name=f"po{nt}", tag="po", bufs=2)
                for c in range(2):
                    pp = PS.tile(
                        [128, 512], f32, name=f"pp{nt}_{c}",
                        tag="work", bufs=2
                    )
                    for kq in range(NPAIR):
                        nc.tensor.matmul(
                            pp,
                            lhsT=onormT[kq][:, nt * 128:(nt + 1) * 128],
                            rhs=wo_sb[kq][:, c * 512:(c + 1) * 512],
                            start=(kq == 0),
                            stop=(kq == NPAIR - 1),
                        )
                    if c == 0:
                        nc.scalar.copy(po[:, c * 512:(c + 1) * 512], pp)
                    else:
                        nc.vector.tensor_copy(po[:, c * 512:(c + 1) * 512], pp)
                nc.sync.dma_start(out_d[nt * 128:(nt + 1) * 128, :], po)

    nc.compile()
    return nc


def _shard_inputs(x, ln_gamma, ln_beta, w_qkv, w_out):
    w_eff = (w_qkv * ln_gamma[None, :]).astype(np.float32)
    wqkvT = np.ascontiguousarray(w_eff.T)                   # [1024, 3072] f32
    bias = (w_qkv.astype(np.float64) @ ln_beta.astype(np.float64)).astype(
        np.float32
    )                                                        # [3072]
    woutT = np.ascontiguousarray(w_out.T)                    # [1024, 1024] f32
    INNER = HEADS * DH

    in_maps = []
    for c in range(NCORES):
        b, hg = c // 2, c % 2
        xb = np.ascontiguousarray(np.asarray(x[b], dtype=np.float32))
        # prepack QKV weights: row-tile (j, ob) holds the [128 feat x
        # 128 out] blocks for all 8 feature k-tiles, contiguous per
        # feature row.
        wpack = np.empty((NOB * 128, D), dtype=bfloat16)
        bias_2d = np.empty((128, NOB), dtype=np.float32)
        for j in range(NPAIR):
            for obi in range(3):
                colbase = obi * INNER + hg * 512 + j * 128
                blk = wqkvT[:, colbase:colbase + 128]        # [1024, 128]
                # dest[p, k*128 + c] = blk[k*128 + p, c]
                r0 = (j * 3 + obi) * 128
                wpack[r0:r0 + 128, :] = (
                    blk.reshape(KD, 128, 128)
                    .transpose(1, 0, 2)
                    .reshape(128, D)
                    .astype(bfloat16)
                )
                bias_2d[:, j * 3 + obi] = bias[colbase:colbase + 128]
        wopack = np.ascontiguousarray(
            woutT[hg * 512:(hg + 1) * 512, :]
        ).astype(bfloat16)                                   # [512, 1024]
        in_maps.append({
            "x": xb,
            "wpack": wpack,
            "qkv_bias": bias_2d,
            "wopack": wopack,
        })
    return in_maps


def kernel(x, ln_gamma, ln_beta, w_qkv, w_out, b_out, _trace=False):
    from concourse import bass_utils

    x = np.asarray(x, dtype=np.float32)
    ln_gamma = np.asarray(ln_gamma, dtype=np.float32)
    ln_beta = np.asarray(ln_beta, dtype=np.float32)
    w_qkv = np.asarray(w_qkv, dtype=np.float32)
    w_out = np.asarray(w_out, dtype=np.float32)
    b_out = np.asarray(b_out, dtype=np.float32)

    if "nc" not in _cache:
        _cache["nc"] = _build()
    nc = _cache["nc"]

    in_maps = _shard_inputs(x, ln_gamma, ln_beta, w_qkv, w_out)
    res = bass_utils.run_bass_kernel_spmd(
        nc, in_maps, core_ids=list(range(NCORES)), trace=_trace
    )
    out = np.empty((B, N, D), dtype=np.float32)
    for b in range(B):
        out[b] = np.asarray(res.results[2 * b]["out"])
        out[b] += np.asarray(res.results[2 * b + 1]["out"])
    out += b_out[None, None, :]
    _cache["last_result"] = res
    return out


# revision 16
# speedup vs baseline: 1.1249x; 1.1249x over previous
"""Trainium2 Bass kernel for a pre-LN multi-head attention block.

Model (per batch b): LayerNorm(x) -> QKV -> 16-head attention (dh=64) ->
output projection + bias.

Sharding over 8 NeuronCores: core c handles batch b = c//2 and head
group hg = c%2 (8 of the 16 heads, all 2048 query rows, full 2048
keys).  LN is duplicated across the pair; QKV/attention are disjoint
per head group; the output projection contracts only this core's 512
inner dims, so each core produces a PARTIAL output and the host adds
the pair's two partials (plus b_out).  No K/V duplication and no
device collectives.

Device-side layout notes:
 - Activations are kept transposed (feature dim on partitions); every
   matmul contracts over the partition axis.
 - Scores are computed directly as S^T [nk, nq] with the two heads of
   a pair row-packed via tile_position (concurrent in the PE array);
   softmax needs no max subtraction (scores ~ N(0,1)), so exp is one
   ScalarE pass and the denominator rides along as a ones-column in
   the PV matmul (M=65).
 - Softmax denominators are collected into one [8, 512] tile per head
   pair and inverted with a single reciprocal_approx_fast (the naive
   per-row DVE reciprocal was ~70us of the baseline).
 - LN: sum(x) on DVE tensor_reduce, sum(x^2) on ScalarE Square+accum
   (both from raw x, var = E[x^2]-mu^2), rstd via
   reciprocal_approx_fast, and (x-mu)*rstd fused in one tensor_scalar.
 - ln_gamma is folded into the QKV weights host-side; the ln_beta
   contribution is a per-output-row bias added at QKV eviction; b_out
   is added host-side.
 - Weights are host-prepacked so each (pair, q/k/v) weight block is a
   single contiguous 256KB DMA (the baseline's 192 small weight DMAs
   were serializing the sync queue).
 - The scores->exp->PV chain is software-pipelined (PV lags one kt)
   with next-pair QKV matmuls interleaved so the PE never waits on
   ScalarE's exp.
"""

import numpy as np
from ml_dtypes import bfloat16

B, N, D = 4, 2048, 1024
HEADS, DH = 16, 64
SCALE = DH ** -0.5
NCORES = 8
NQ = N                      # all 2048 query rows per core
EPS = 1e-5
NT = N // 128               # 16 sequence tiles (LN)
KD = D // 128               # 8 feature tiles
NKT = N // 128              # 16 key tiles
NPAIR = 4                   # head pairs per core (8 heads)
NBLK = NQ // 512            # 4 query blocks of 512
NOB = 12                    # (pair, q/k/v) output row-tiles

_cache = {}


def _build():
    import concourse.bass as bass
    import concourse.mybir as mybir
    import concourse.bacc as bacc
    import concourse.tile as tile
    from concourse.masks import make_identity

    f32 = mybir.dt.float32
    bf16 = mybir.dt.bfloat16
    AX = mybir.AxisListType
    ALU = mybir.AluOpType
    ACTF = mybir.ActivationFunctionType

    nc = bacc.Bacc(
        "TRN2",
        target_bir_lowering=False,
        debug=False,
        enable_asserts=True,
        num_devices=NCORES,
    )

    x_d = nc.dram_tensor("x", [N, D], f32, kind="ExternalInput").ap()
    wq_d = nc.dram_tensor("wpack", [NOB * 128, D], bf16, kind="ExternalInput").ap()
    bias_d = nc.dram_tensor("qkv_bias", [128, NOB], f32, kind="ExternalInput").ap()
    wo_d = nc.dram_tensor("wopack", [NPAIR * 128, D], bf16, kind="ExternalInput").ap()
    out_d = nc.dram_tensor("out", [NQ, D], f32, kind="ExternalOutput").ap()

    with tile.TileContext(nc) as tc:
        with (
            tc.tile_pool(name="persist", bufs=1) as P,
            tc.tile_pool(name="ppool", bufs=1, space="PSUM") as PS,
            tc.tile_pool(name="trans", bufs=1) as T,
        ):
            ident = P.tile([128, 128], bf16, name="ident", tag="ident")
            make_identity(nc, ident)
            eps_t = P.tile([128, 1], f32, name="eps_t", tag="eps_t")
            nc.vector.memset(eps_t, EPS)

            bias_sb = P.tile([128, NOB], f32, name="bias_sb", tag="bias_sb")
            nc.sync.dma_start(bias_sb, bias_d)

            wo_sb = []

            # xnT: transposed normalized activations [d, n] as [128, KD*N]
            xnT = P.tile([128, KD * N], bf16, name="xnT", tag="xnT")
            xnT3 = xnT.rearrange("p (k n) -> p k n", k=KD)
            # normalized attention outputs, transposed: [512 hd, 2048 nq]
            onormT = []
            for k in range(NPAIR):
                onormT.append(
                    P.tile([128, NQ], bf16, name=f"onormT{k}", tag=f"onormT{k}")
                )
            # persistent V_ext buffers [nk, dh | ones]: 4 rotating, the
            # ones-column (idx 64) is initialized once and never rewritten
            ve_bufs = []
            for i in range(4):
                ve = P.tile([128, NKT * 65], bf16, name=f"vx{i}", tag=f"vx{i}")
                ve3 = ve.rearrange("p (k e) -> p k e", e=65)
                nc.vector.memset(ve3[:, :, 64:65], 1.0)
                ve_bufs.append(ve3)

            sq_scr = T.tile([128, D], f32, name="sq_scr", tag="sq", bufs=1)

            # QKV projection work for pair j, packaged as emission
            # closures so it can be interleaved into LN (pair 0) or the
            # previous pair's attention loop.
            def qkv_emitters(j, store):
                ems = []
                reqs = []
                for obi, (key, _) in enumerate(
                    (("qT", 0), ("kT", 1), ("vT", 2))
                ):
                    idx = j * 3 + obi

                    def walloc(j=j, idx=idx, key=key):
                        wt = T.tile(
                            [128, D], bf16, name=f"w{key}{j}",
                            tag=f"w{key}", bufs=2,
                        )
                        nc.sync.dma_start(
                            wt, wq_d[idx * 128:(idx + 1) * 128, :]
                        )
                        store[(key, "w")] = wt
                        store[key] = T.tile(
                            [128, N], bf16, name=f"t{key}{j}", tag=key,
                            bufs=2,
                        )
                    ems.append(walloc)
                    reqs.append(-1)
                    for c in range(N // 512):
                        def chunk(j=j, c=c, idx=idx, key=key):
                            qp = PS.tile(
                                [128, 512], f32, name=f"qp{key}{j}_{c}",
                                tag="work", bufs=2,
                            )
                            wt = store[(key, "w")]
                            for k in range(KD):
                                nc.tensor.matmul(
                                    qp,
                                    lhsT=wt[:, k * 128:(k + 1) * 128],
                                    rhs=xnT3[:, k, c * 512:(c + 1) * 512],
                                    start=(k == 0),
                                    stop=(k == KD - 1),
                                )
                            dcol = store[key][:, c * 512:(c + 1) * 512]
                            nc.vector.tensor_scalar_add(
                                dcol, qp, bias_sb[:, idx:idx + 1]
                            )
                        ems.append(chunk)
                        reqs.append(4 * c + 3)
                # V_ext for the two heads: [nk, dh] blocks per key tile
                for h2 in range(2):
                    for g2 in range(2):
                        def vtr(j=j, h2=h2, g2=g2):
                            p0 = h2 * 64
                            id64 = ident[p0:p0 + 64, p0:p0 + 64]
                            vT_j = store["vT"]
                            ve3 = ve_bufs[(2 * j + h2) % 4]
                            tp = PS.tile(
                                [128, 512], bf16, name=f"vt{j}_{h2}_{g2}",
                                tag="work", bufs=2,
                            )
                            for i8 in range(8):
                                kt = g2 * 8 + i8
                                nc.tensor.transpose(
                                    tp[:, i8 * 64:(i8 + 1) * 64],
                                    vT_j[p0:p0 + 64, kt * 128:(kt + 1) * 128],
                                    id64,
                                )
                            dest = ve3[:, g2 * 8:(g2 + 1) * 8, 0:64]
                            src = tp.rearrange("p (k e) -> p k e", e=64)
                            nc.vector.tensor_copy(dest, src)
                        ems.append(vtr)
                        reqs.append(8 * g2 + 7)
                return ems, reqs

            stores = [dict() for _ in range(NPAIR + 1)]
            # pair-0 weight DMAs go on the sync queue BEFORE the LN x
            # loads so they aren't stuck behind x-buffer WAR waits
            ems0, reqs0 = qkv_emitters(0, stores[0])
            order0 = sorted(range(len(ems0)), key=lambda i: reqs0[i])
            ems0 = [ems0[i] for i in order0]
            reqs0 = [reqs0[i] for i in order0]
            e0i = 0
            while e0i < len(ems0) and reqs0[e0i] < 0:
                ems0[e0i]()
                e0i += 1

            # ---- Phase A: LayerNorm + transpose, two passes over seq
            # tiles so ScalarE's Square and Sqrt calls are batched by ACT
            # table set (each Square<->Sqrt switch costs a 1.3us
            # ACT_TABLE_LOAD; the interleaved version paid it 13x).
            # Pass 1: stats + centered xc (bf16); pass 2: sqrt + scale +
            # transpose.
            for wave in range(2):
                ln_state = {}
                # all 8 vareps land in one [128, 8] tile so the wave's
                # sqrt is a single ACT instruction — the tile scheduler
                # can then never interleave Square/Sqrt table sets
                vareps_w = T.tile([128, 8], f32, name=f"vew{wave}",
                                  tag="vep", bufs=2)
                for i, nt in enumerate(range(wave * 8, wave * 8 + 8)):
                    x_t = T.tile([128, D], f32, name=f"x{nt}", tag="x",
                                 bufs=3)
                    nc.sync.dma_start(x_t, x_d[nt * 128:(nt + 1) * 128, :])
                    ssum = T.tile([128, 1], f32, name=f"ss{nt}", tag="ss",
                                  bufs=3)
                    nc.vector.tensor_reduce(ssum, x_t, AX.X, ALU.add)
                    negmean = T.tile([128, 1], f32, name=f"nm{nt}", tag="nm",
                                     bufs=3)
                    nc.scalar.mul(negmean, ssum, -1.0 / D)
                    # varsum = sum((x - mu)^2) via the ACT bias port
                    varsum = T.tile([128, 1], f32, name=f"vs{nt}", tag="vs",
                                    bufs=3)
                    nc.scalar.activation(sq_scr, x_t, ACTF.Square,
                                         bias=negmean, accum_out=varsum)
                    nc.vector.tensor_scalar(
                        vareps_w[:, i:i + 1], varsum, 1.0 / D, EPS,
                        ALU.mult, ALU.add
                    )
                    xc = T.tile([128, D], bf16, name=f"xc{nt}", tag="xc",
                                bufs=9)
                    nc.vector.tensor_scalar_add(xc, x_t, negmean)
                    ln_state[nt] = xc
                std_w = T.tile([128, 8], f32, name=f"stw{wave}", tag="st",
                               bufs=2)
                nc.scalar.sqrt(std_w, vareps_w)
                rstd_w = T.tile([128, 8], f32, name=f"rsw{wave}", tag="rs",
                                bufs=2)
                nc.vector.reciprocal_approx_fast(rstd_w, std_w)
                for i, nt in enumerate(range(wave * 8, wave * 8 + 8)):
                    xc = ln_state[nt]
                    xhat = T.tile([128, D], bf16, name=f"xh{nt}", tag="xh",
                                  bufs=3)
                    nc.vector.tensor_scalar_mul(xhat, xc, rstd_w[:, i:i + 1])
                    for g2 in range(2):
                        tp = PS.tile(
                            [128, 512], bf16, name=f"tp{nt}_{g2}",
                            tag="work", bufs=2
                        )
                        for jj in range(4):
                            kd = g2 * 4 + jj
                            nc.tensor.transpose(
                                tp[:, jj * 128:(jj + 1) * 128],
                                xhat[:, kd * 128:(kd + 1) * 128],
                                ident,
                            )
                        dest = xnT3[:, g2 * 4:(g2 + 1) * 4,
                                    nt * 128:(nt + 1) * 128]
                        src = tp.rearrange("p (k n) -> p k n", k=4)
                        if (nt + g2) % 2 == 0:
                            nc.vector.tensor_copy(dest, src)
                        else:
                            nc.scalar.copy(dest, src)
                    # interleave pair-0 QKV emission once its LN inputs
                    # have been emitted (Tile deps follow emission order)
                    while e0i < len(ems0) and reqs0[e0i] <= nt:
                        ems0[e0i]()
                        e0i += 1

            # ---- Phases B+C: per head pair, attention row-packed via
            # tile_position so K stays covered.
            while e0i < len(ems0):
                ems0[e0i]()
                e0i += 1
            for j in range(NPAIR):
                st = stores[j]
                if j == 2:
                    for k in range(NPAIR):
                        t = P.tile([128, D], bf16, name=f"wo{k}", tag=f"wo{k}")
                        nc.sync.dma_start(t, wo_d[k * 128:(k + 1) * 128, :])
                        wo_sb.append(t)
                qT_j, kT_j = st["qT"], st["kT"]
                ve3s = [ve_bufs[(2 * j) % 4], ve_bufs[(2 * j + 1) % 4]]
                if j + 1 < NPAIR:
                    pe_, pr_ = qkv_emitters(j + 1, stores[j + 1])
                    po_ = sorted(range(len(pe_)), key=lambda i: pr_[i])
                    pend = [pe_[i] for i in po_]
                else:
                    pend = []
                pi = 0
                norm_defer = []
                for blk in range(NBLK):
                    b0 = blk * 512
                    opss = [
                        PS.tile([65, 512], f32, name=f"ops{2*j}_{blk}",
                                tag="acc0", bufs=1),
                        PS.tile([65, 512], f32, name=f"ops{2*j+1}_{blk}",
                                tag="acc1", bufs=1),
                    ]
                    pts = [None, None]
                    for kt in range(NKT + 1):
                        if kt < NKT:
                            sps = PS.tile(
                                [128, 1024], f32, name=f"s{j}_{blk}_{kt}",
                                tag="spair", bufs=2,
                            )
                            for h2 in range(2):
                                p0 = h2 * 64
                                nc.tensor.matmul(
                                    sps[:, h2 * 512:(h2 + 1) * 512],
                                    lhsT=kT_j[p0:p0 + 64,
                                              kt * 128:(kt + 1) * 128],
                                    rhs=qT_j[p0:p0 + 64, b0:b0 + 512],
                                    start=True,
                                    stop=True,
                                    tile_position=(p0, 0),
                                )
                            pt = T.tile(
                                [128, 1024], bf16, name=f"pt{j}_{blk}_{kt}",
                                tag="pt", bufs=4,
                            )
                            nc.scalar.activation(pt, sps, ACTF.Exp, scale=SCALE)
                            pts[kt % 2] = pt
                        # interleave next-pair QKV emission across kt slots
                        it = blk * (NKT + 1) + kt + 1
                        tot = NBLK * (NKT + 1)
                        while pi < len(pend) and pi * tot < len(pend) * it:
                            pend[pi]()
                            pi += 1
                        # PV lags one kt so exp(kt) never blocks the PE
                        if kt >= 1:
                            ptp = pts[(kt - 1) % 2]
                            for h2 in range(2):
                                nc.tensor.matmul(
                                    opss[h2],
                                    lhsT=ve3s[h2][:, kt - 1, :],
                                    rhs=ptp[:, h2 * 512:(h2 + 1) * 512],
                                    start=(kt - 1 == 0),
                                    stop=(kt - 1 == NKT - 1),
                                )
                    # evict accumulators fast (frees the psum bank); the
                    # denominator row goes to a partition-0 tile so the
                    # fast approx reciprocal can be used (custom DVE ops
                    # misread partition-offset inputs)
                    for h2 in range(2):
                        r = 2 * blk + h2
                        oc = T.tile([64, 512], f32, name=f"oc{j}_{r}",
                                    tag="oc", bufs=8)
                        nc.vector.tensor_copy(oc, opss[h2][0:64, :])
                        rl0 = T.tile([1, 512], f32, name=f"rl0{j}_{r}",
                                     tag="rl0", bufs=8)
                        nc.vector.tensor_copy(rl0, opss[h2][64:65, :])
                        norm_defer.append((h2, blk, oc, rl0))
                while pi < len(pend):
                    pend[pi]()
                    pi += 1
                for h2, blk, oc, rl0 in norm_defer:
                    r = 2 * blk + h2
                    b0 = blk * 512
                    p0 = h2 * 64
                    rl = T.tile([1, 512], f32, name=f"rl{j}_{r}",
                                tag="rl", bufs=2)
                    nc.vector.reciprocal_approx_fast(rl, rl0)
                    rlb = T.tile([64, 512], f32, name=f"rlb{j}_{r}",
                                 tag="rlb", bufs=2)
                    nc.gpsimd.partition_broadcast(rlb, rl, channels=64)
                    nc.vector.tensor_mul(
                        onormT[j][p0:p0 + 64, b0:b0 + 512], oc, rlb
                    )
                norm_defer = []

            # ---- Phase D: output projection [2048 nq, 1024 dm] (partial:
            # this core's 512 inner dims; host adds the pair's partials)
            for nt in range(NQ // 128):
                po = T.tile([128, D], f32, name=f"po{nt}", tag="po", bufs=2)
                for c in range(2):
                    pp = PS.tile(
                        [128, 512], f32, name=f"pp{nt}_{c}",
                        tag="work", bufs=2
                    )
                    for kq in range(NPAIR):
                        nc.tensor.matmul(
                            pp,
                            lhsT=onormT[kq][:, nt * 128:(nt + 1) * 128],
                            rhs=wo_sb[kq][:, c * 512:(c + 1) * 512],
                            start=(kq == 0),
                            stop=(kq == NPAIR - 1),
                        )
                    if c == 0:
                        nc.scalar.copy(po[:, c * 512:(c + 1) * 512], pp)
                    else:
                        nc.vector.tensor_copy(po[:, c * 512:(c + 1) * 512], pp)
                nc.sync.dma_start(out_d[nt * 128:(nt + 1) * 128, :], po)

    nc.compile()
    return nc


def _shard_inputs(x, ln_gamma, ln_beta, w_qkv, w_out):
    w_eff = (w_qkv * ln_gamma[None, :]).astype(np.float32)
    wqkvT = np.ascontiguousarray(w_eff.T)                   # [1024, 3072] f32
    bias = (w_qkv.astype(np.float64) @ ln_beta.astype(np.float64)).astype(
        np.float32
    )                                                        # [3072]
    woutT = np.ascontiguousarray(w_out.T)                    # [1024, 1024] f32
    INNER = HEADS * DH

    in_maps = []
    for c in range(NCORES):
        b, hg = c // 2, c % 2
        xb = np.ascontiguousarray(np.asarray(x[b], dtype=np.float32))
        # prepack QKV weights: row-tile (j, ob) holds the [128 feat x
        # 128 out] blocks for all 8 feature k-tiles, contiguous per
        # feature row.
        wpack = np.empty((NOB * 128, D), dtype=bfloat16)
        bias_2d = np.empty((128, NOB), dtype=np.float32)
        for j in range(NPAIR):
            for obi in range(3):
                colbase = obi * INNER + hg * 512 + j * 128
                blk = wqkvT[:, colbase:colbase + 128]        # [1024, 128]
                # dest[p, k*128 + c] = blk[k*128 + p, c]
                r0 = (j * 3 + obi) * 128
                wpack[r0:r0 + 128, :] = (
                    blk.reshape(KD, 128, 128)
                    .transpose(1, 0, 2)
                    .reshape(128, D)
                    .astype(bfloat16)
                )
                bias_2d[:, j * 3 + obi] = bias[colbase:colbase + 128]
        wopack = np.ascontiguousarray(
            woutT[hg * 512:(hg + 1) * 512, :]
        ).astype(bfloat16)                                   # [512, 1024]
        in_maps.append({
            "x": xb,
            "wpack": wpack,
            "qkv_bias": bias_2d,
            "wopack": wopack,
        })
    return in_maps


def kernel(x, ln_gamma, ln_beta, w_qkv, w_out, b_out, _trace=False):
    from concourse import bass_utils

    x = np.asarray(x, dtype=np.float32)
    ln_gamma = np.asarray(ln_gamma, dtype=np.float32)
    ln_beta = np.asarray(ln_beta, dtype=np.float32)
    w_qkv = np.asarray(w_qkv, dtype=np.float32)
    w_out = np.asarray(w_out, dtype=np.float32)
    b_out = np.asarray(b_out, dtype=np.float32)

    if "nc" not in _cache:
        _cache["nc"] = _build()
    nc = _cache["nc"]

    in_maps = _shard_inputs(x, ln_gamma, ln_beta, w_qkv, w_out)
    res = bass_utils.run_bass_kernel_spmd(
        nc, in_maps, core_ids=list(range(NCORES)), trace=_trace
    )
    out = np.empty((B, N, D), dtype=np.float32)
    for b in range(B):
        out[b] = np.asarray(res.results[2 * b]["out"])
        out[b] += np.asarray(res.results[2 * b + 1]["out"])
    out += b_out[None, None, :]
    _cache["last_result"] = res
    return out
